# revision 3
# baseline (speedup 1.0000x reference)
"""Two-layer GCN (DGL GraphConv norm='both') on 8 Trainium2 NeuronCores.

Dense-adjacency streaming design (replaces the dma_gather design):
  - Nodes range-partitioned across 8 cores (1250 dst nodes each, 10 windows
    of 128). Host builds, per core, a DENSE adjacency slab per dst window:
    A1[p, w, s*128+d] = multiplicity of edge (src=s*128+p -> dst window w,
    local d), bf16. Aggregation becomes plain TensorE matmuls
        agg[dst, :] += A1[:, w, s-block]^T @ X[s-block, :]
    accumulating over 79 src chunks in PSUM; the A slab (2.6 MB) streams in
    with one large sequential DMA per window and overlaps with compute.
    No dma_gather anywhere.
  - norm_src is folded into the feature table host-side; norm_dst and biases
    are applied per-window with scalar_tensor_tensor; W1 is applied after
    aggregation (norm commutes), then ReLU, then z = (relu * norm_src) @ W2
    is computed locally and AllGathered (bf16, 1280 rows/core).
  - Layer 2 repeats the same dense aggregation over the gathered z with a
    second adjacency A2 indexed in padded-z row order (80 src chunks).
"""

import numpy as np
import ml_dtypes

BF16 = ml_dtypes.bfloat16
N_CORES = 8
N, F, H, C = 10000, 256, 256, 64
NPC = N // N_CORES          # 1250 dst nodes per core
NT = 10                     # dst windows per core (10*128 = 1280 >= 1250)
KS1 = 79                    # src chunks of 128 over nodes (79*128 = 10112)
KS2 = 80                    # src chunks over padded z rows (8*1280 = 10240)

LAST_STATS = {}


def _prep(features, W1, b1, W2, b2, src, dst):
    src = np.asarray(src, np.int64)
    dst = np.asarray(dst, np.int64)
    out_deg = np.bincount(src, minlength=N).astype(np.float32)
    in_deg = np.bincount(dst, minlength=N).astype(np.float32)
    ns = (1.0 / np.sqrt(np.clip(out_deg, 1.0, None))).astype(np.float32)
    nd = (1.0 / np.sqrt(np.clip(in_deg, 1.0, None))).astype(np.float32)

    featb = (np.asarray(features, np.float32) * ns[:, None]).astype(BF16)
    xpad = np.zeros((KS1 * 128, F), BF16)
    xpad[:N] = featb
    # xsb[p, s*F+f] = xpad[s*128+p, f]  (contiguous per-partition DMA)
    xsb = np.ascontiguousarray(
        xpad.reshape(KS1, 128, F).transpose(1, 0, 2)).reshape(128, KS1 * F)

    # W1 chunked for lhsT-free rhs use: w1t[p, k*H+n] = W1[k*128+p, n]
    w1t = np.ascontiguousarray(
        np.asarray(W1, np.float32).astype(BF16).reshape(2, 128, H)
        .transpose(1, 0, 2)).reshape(128, 2 * H)
    w2t = np.ascontiguousarray(
        np.asarray(W2, np.float32).astype(BF16).reshape(2, 128, C)
        .transpose(1, 0, 2)).reshape(128, 2 * C)
    b1f = np.asarray(b1, np.float32)[None, :]
    b2f = np.asarray(b2, np.float32)[None, :]

    in_maps = []
    for c in range(N_CORES):
        m = (dst >= c * NPC) & (dst < (c + 1) * NPC)
        s, dl = src[m], dst[m] - c * NPC
        w, dloc = dl // 128, dl % 128
        a1 = np.zeros((128, NT, KS1 * 128), np.float32)
        np.add.at(a1, (s % 128, w, (s // 128) * 128 + dloc), 1.0)
        sz = (s // NPC) * 1280 + (s % NPC)
        a2 = np.zeros((128, NT, KS2 * 128), np.float32)
        np.add.at(a2, (sz % 128, w, (sz // 128) * 128 + dloc), 1.0)

        ndst_t = np.ones((128, NT), np.float32)
        nso_t = np.ones((128, NT), np.float32)
        loc = np.arange(NPC)
        ndst_t[loc % 128, loc // 128] = nd[c * NPC:(c + 1) * NPC]
        nso_t[loc % 128, loc // 128] = ns[c * NPC:(c + 1) * NPC]

        in_maps.append(dict(
            a1=a1.astype(BF16), a2=a2.astype(BF16), xsb=xsb,
            ndst=ndst_t, nso=nso_t, w1=w1t, w2=w2t, b1=b1f, b2=b2f,
        ))
    return in_maps


def _build(variant="full", num_devices=N_CORES):
    import concourse.bacc as bacc
    import concourse.mybir as mybir
    from concourse import tile

    dt = mybir.dt
    alu = mybir.AluOpType

    nc = bacc.Bacc("TRN2", target_bir_lowering=False, debug=False,
                   num_devices=num_devices, num_swdge_queues=4)

    a1_d = nc.dram_tensor("a1", [128, NT, KS1 * 128], dt.bfloat16,
                          kind="ExternalInput")
    a2_d = nc.dram_tensor("a2", [128, NT, KS2 * 128], dt.bfloat16,
                          kind="ExternalInput")
    xsb_d = nc.dram_tensor("xsb", [128, KS1 * F], dt.bfloat16,
                           kind="ExternalInput")
    ndst_d = nc.dram_tensor("ndst", [128, NT], dt.float32, kind="ExternalInput")
    nso_d = nc.dram_tensor("nso", [128, NT], dt.float32, kind="ExternalInput")
    w1_d = nc.dram_tensor("w1", [128, 2 * H], dt.bfloat16, kind="ExternalInput")
    w2_d = nc.dram_tensor("w2", [128, 2 * C], dt.bfloat16, kind="ExternalInput")
    b1_d = nc.dram_tensor("b1", [1, H], dt.float32, kind="ExternalInput")
    b2_d = nc.dram_tensor("b2", [1, C], dt.float32, kind="ExternalInput")
    out_d = nc.dram_tensor("out", [NT * 128, C], dt.float32,
                           kind="ExternalOutput")

    with tile.TileContext(nc) as tc:
        with (
            tc.tile_pool(name="const", bufs=1) as const,
            tc.tile_pool(name="dram", bufs=1, space="DRAM") as dram,
            tc.tile_pool(name="ap", bufs=3) as apool,
            tc.tile_pool(name="work", bufs=3) as work,
            tc.tile_pool(name="ps_agg", bufs=2, space="PSUM") as ps_agg,
            tc.tile_pool(name="ps_tr", bufs=2, space="PSUM") as ps_tr,
            tc.tile_pool(name="ps_h", bufs=2, space="PSUM") as ps_h,
        ):
            # ---- constants ----
            xsb = const.tile([128, KS1 * F], dt.bfloat16)
            nc.sync.dma_start(xsb[:], xsb_d.ap())
            ndst_sb = const.tile([128, NT], dt.float32)
            nc.sync.dma_start(ndst_sb[:], ndst_d.ap())
            nso_sb = const.tile([128, NT], dt.float32)
            nc.sync.dma_start(nso_sb[:], nso_d.ap())
            w1_sb = const.tile([128, 2 * H], dt.bfloat16)
            nc.sync.dma_start(w1_sb[:], w1_d.ap())
            w2_sb = const.tile([128, 2 * C], dt.bfloat16)
            nc.sync.dma_start(w2_sb[:], w2_d.ap())
            b1_sb = const.tile([1, H], dt.float32)
            nc.sync.dma_start(b1_sb[:], b1_d.ap())
            b2_sb = const.tile([1, C], dt.float32)
            nc.sync.dma_start(b2_sb[:], b2_d.ap())

            iota_bf = const.tile([128, 128], dt.bfloat16)
            nc.gpsimd.iota(iota_bf[:], pattern=[[1, 128]], base=0,
                           channel_multiplier=0,
                           allow_small_or_imprecise_dtypes=True)
            iota_col = const.tile([128, 1], dt.float32)
            nc.gpsimd.iota(iota_col[:], pattern=[[0, 1]], base=0,
                           channel_multiplier=1,
                           allow_small_or_imprecise_dtypes=True)
            ident_bf = const.tile([128, 128], dt.bfloat16)
            nc.vector.tensor_scalar(ident_bf[:], iota_bf[:], iota_col[:],
                                    None, alu.is_equal)

            # bias rows broadcast across partitions via ones-column matmul
            ones_sb = const.tile([1, 128], dt.float32)
            nc.vector.memset(ones_sb[:], 1.0)
            b1_ps = ps_h.tile([128, H], dt.float32, tag="h")
            nc.tensor.matmul(b1_ps[:], lhsT=ones_sb[:], rhs=b1_sb[:],
                             start=True, stop=True)
            b1_bc = const.tile([128, H], dt.float32)
            nc.vector.tensor_copy(b1_bc[:], b1_ps[:])
            b2_ps = ps_h.tile([128, C], dt.float32, tag="zn")
            nc.tensor.matmul(b2_ps[:], lhsT=ones_sb[:], rhs=b2_sb[:],
                             start=True, stop=True)
            b2_bc = const.tile([128, C], dt.float32)
            nc.vector.tensor_copy(b2_bc[:], b2_ps[:])

            zsb = const.tile([128, KS2, C], dt.bfloat16)
            cc_in = dram.tile([NT * 128, C], dt.bfloat16)
            z_full = dram.tile([N_CORES * NT * 128, C], dt.bfloat16,
                               addr_space="Shared")

            # ---- layer 1 ----
            for w in range(NT):
                a1 = apool.tile([128, KS2 * 128], dt.bfloat16, tag="a")
                if variant == "no_l1dma":
                    nc.vector.memset(a1[:, 0:KS1 * 128], 0.25)
                else:
                    nc.sync.dma_start(a1[:, 0:KS1 * 128], a1_d.ap()[:, w, :])
                agg = ps_agg.tile([128, F], dt.float32, tag="agg")
                for s in range(KS1):
                    nc.tensor.matmul(agg[:],
                                     lhsT=a1[:, s * 128:(s + 1) * 128],
                                     rhs=xsb[:, s * F:(s + 1) * F],
                                     start=(s == 0), stop=(s == KS1 - 1))
                aggc = work.tile([128, F], dt.bfloat16, tag="aggc")
                nc.vector.tensor_copy(aggc[:], agg[:])
                aggT = work.tile([128, 2, 128], dt.bfloat16, tag="aggT")
                for k in range(2):
                    trp = ps_tr.tile([128, 128], dt.bfloat16, tag="tr")
                    nc.tensor.transpose(trp[:], aggc[:, k * 128:(k + 1) * 128],
                                        ident_bf[:])
                    nc.vector.tensor_copy(aggT[:, k, :], trp[:])
                h1 = ps_h.tile([128, H], dt.float32, tag="h")
                for k in range(2):
                    nc.tensor.matmul(h1[:], lhsT=aggT[:, k, :],
                                     rhs=w1_sb[:, k * H:(k + 1) * H],
                                     start=(k == 0), stop=(k == 1))
                t1 = work.tile([128, H], dt.float32, tag="t1")
                nc.vector.scalar_tensor_tensor(t1[:], h1[:], ndst_sb[:, w:w + 1],
                                               b1_bc[:], alu.mult, alu.add)
                yz = work.tile([128, H], dt.bfloat16, tag="yz")
                nc.scalar.activation(yz[:], t1[:],
                                     mybir.ActivationFunctionType.Relu,
                                     scale=nso_sb[:, w:w + 1])
                yzT = work.tile([128, 2, 128], dt.bfloat16, tag="yzT")
                for k in range(2):
                    trp2 = ps_tr.tile([128, 128], dt.bfloat16, tag="tr")
                    nc.tensor.transpose(trp2[:], yz[:, k * 128:(k + 1) * 128],
                                        ident_bf[:])
                    nc.vector.tensor_copy(yzT[:, k, :], trp2[:])
                zn = ps_h.tile([128, C], dt.float32, tag="zn")
                for k in range(2):
                    nc.tensor.matmul(zn[:], lhsT=yzT[:, k, :],
                                     rhs=w2_sb[:, k * C:(k + 1) * C],
                                     start=(k == 0), stop=(k == 1))
                znb = work.tile([128, C], dt.bfloat16, tag="znb")
                nc.vector.tensor_copy(znb[:], zn[:])
                nc.sync.dma_start(cc_in[w * 128:(w + 1) * 128, :], znb[:])

            # ---- halo exchange ----
            if variant == "no_cc" or num_devices == 1:
                for i in range(N_CORES):
                    nc.sync.dma_start(
                        z_full[i * NT * 128:(i + 1) * NT * 128, :], cc_in[:, :])
            else:
                nc.gpsimd.collective_compute(
                    "AllGather", alu.bypass,
                    replica_groups=[list(range(N_CORES))],
                    ins=[cc_in.opt()], outs=[z_full.opt()])

            nc.sync.dma_start(
                zsb[:], z_full[:, :].rearrange("(k p) n -> p k n", p=128))

            # ---- layer 2 ----
            for w in range(NT):
                a2 = apool.tile([128, KS2 * 128], dt.bfloat16, tag="a")
                if variant == "no_l2dma":
                    nc.vector.memset(a2[:], 0.25)
                else:
                    nc.sync.dma_start(a2[:], a2_d.ap()[:, w, :])
                agg2 = ps_agg.tile([128, C], dt.float32, tag="agg")
                for s in range(KS2):
                    nc.tensor.matmul(agg2[:],
                                     lhsT=a2[:, s * 128:(s + 1) * 128],
                                     rhs=zsb[:, s, :],
                                     start=(s == 0), stop=(s == KS2 - 1))
                ot = work.tile([128, C], dt.float32, tag="ot")
                nc.vector.scalar_tensor_tensor(ot[:], agg2[:],
                                               ndst_sb[:, w:w + 1], b2_bc[:],
                                               alu.mult, alu.add)
                nc.sync.dma_start(out_d.ap()[w * 128:(w + 1) * 128, :], ot[:])

    nc.compile()
    return nc


def kernel(features, W1, b1, W2, b2, src, dst, **_):
    import time
    from concourse.bass_utils import run_bass_kernel_spmd

    t0 = time.time()
    in_maps = _prep(features, W1, b1, W2, b2, src, dst)
    t1 = time.time()
    nc = _build()
    t2 = time.time()
    res = run_bass_kernel_spmd(nc, in_maps, core_ids=list(range(N_CORES)))
    t3 = time.time()
    out = np.concatenate([res.results[c]["out"][:NPC] for c in range(N_CORES)], 0)
    LAST_STATS.update(prep_s=t1 - t0, build_s=t2 - t1, run_s=t3 - t2)
    return np.ascontiguousarray(out.astype(np.float32))


# revision 9
# speedup vs baseline: 154.0602x; 154.0602x over previous
"""Two-layer GCN (DGL GraphConv norm='both') on 8 Trainium2 NeuronCores.

Minimal-input design: the per-call cost in this harness is dominated by
host->device input staging, so inputs are kept tiny (~1.2 MB/core):
  - Features are SHARDED (1250 rows/core, norm_src folded, bf16) and
    AllGathered on device into the full table.
  - The graph ships as per-block edge id lists: nodes are range-partitioned
    (1250 dst/core = 10 windows of 128); the padded global node space
    (8 cores x 1280 rows = 80 chunks of 128) gives an (NT x 80) block grid
    per core. Each block's edges (dst in window w, src in chunk s) are
    listed as (src_local, dst_local) pairs padded to 128 with -1.
  - On device, per window, two bulk DVE is_equal ops turn the id lists into
    one-hot matrices [128e x 128]; per block a TensorE matmul
    onehotS^T @ onehotD builds the dense 128x128 adjacency tile A_T
    (multiplicity counts), which feeds the aggregation matmul
    agg[dst,:] += A_T^T @ X[chunk s].  A_T tiles are built 4-per-PSUM-bank
    and copied to SBUF in bulk (alternating DVE/ACT).
  - Layer tail: h1 = (agg @ W1)*norm_dst + b1, ReLU*norm_src, @ W2 -> z
    (bf16), AllGather z, then layer 2 repeats the same block scheme with
    rhs = z (identical id lists / block grid thanks to padded indexing).
  - Shards ship pre-interleaved (row p*10+k = local row k*128+p) so the
    post-AllGather SBUF chunk loads are 8 clean contiguous DMAs per table.
"""

import numpy as np
import ml_dtypes

BF16 = ml_dtypes.bfloat16
N_CORES = 8
N, F, H, C = 10000, 256, 256, 64
NPC = N // N_CORES          # 1250 dst nodes per core
NT = 10                     # dst windows per core (10*128 = 1280)
KS = 80                     # src chunks over padded node space (8*1280/128)
NBLK = NT * KS              # 800 blocks per core

LAST_STATS = {}


def _prep(features, W1, b1, W2, b2, src, dst):
    src = np.asarray(src, np.int64)
    dst = np.asarray(dst, np.int64)
    out_deg = np.bincount(src, minlength=N).astype(np.float32)
    in_deg = np.bincount(dst, minlength=N).astype(np.float32)
    ns = (1.0 / np.sqrt(np.clip(out_deg, 1.0, None))).astype(np.float32)
    nd = (1.0 / np.sqrt(np.clip(in_deg, 1.0, None))).astype(np.float32)

    featb = (np.asarray(features, np.float32) * ns[:, None]).astype(BF16)

    # padded z/node row space: node n -> row (n//NPC)*1280 + n%NPC
    src_z = (src // NPC) * 1280 + (src % NPC)

    w1t = np.ascontiguousarray(
        np.asarray(W1, np.float32).astype(BF16).reshape(2, 128, H)
        .transpose(1, 0, 2)).reshape(128, 2 * H)
    w2t = np.ascontiguousarray(
        np.asarray(W2, np.float32).astype(BF16).reshape(2, 128, C)
        .transpose(1, 0, 2)).reshape(128, 2 * C)
    b1f = np.asarray(b1, np.float32)[None, :]
    b2f = np.asarray(b2, np.float32)[None, :]

    in_maps = []
    for c in range(N_CORES):
        m = (dst >= c * NPC) & (dst < (c + 1) * NPC)
        s_z, dl = src_z[m], dst[m] - c * NPC
        w, dloc = dl // 128, dl % 128
        schunk, sloc = s_z // 128, s_z % 128
        blk = w * KS + schunk
        order = np.argsort(blk, kind="stable")
        blk, sloc, dloc = blk[order], sloc[order], dloc[order]
        # position of each edge within its block
        pos = np.arange(blk.size) - np.searchsorted(blk, blk)
        if blk.size and pos.max() >= 128:
            raise AssertionError(f"block overflow: {pos.max() + 1} edges")
        ids_s = np.full((128, NBLK), -1.0, np.float32)
        ids_d = np.full((128, NBLK), -1.0, np.float32)
        ids_s[pos, blk] = sloc
        ids_d[pos, blk] = dloc

        # feature shard, pre-interleaved: row p*NT+k = local row k*128+p
        xs = np.zeros((NT * 128, F), BF16)
        xs[:NPC] = featb[c * NPC:(c + 1) * NPC]
        xshard = np.ascontiguousarray(
            xs.reshape(NT, 128, F).transpose(1, 0, 2)).reshape(NT * 128, F)

        ndst_t = np.ones((128, NT), np.float32)
        nso_t = np.ones((128, NT), np.float32)
        loc = np.arange(NPC)
        ndst_t[loc % 128, loc // 128] = nd[c * NPC:(c + 1) * NPC]
        nso_t[loc % 128, loc // 128] = ns[c * NPC:(c + 1) * NPC]

        in_maps.append(dict(
            ids_s=ids_s.astype(BF16), ids_d=ids_d.astype(BF16),
            xshard=xshard, ndst=ndst_t, nso=nso_t,
            w1=w1t, w2=w2t, b1=b1f, b2=b2f,
        ))
    return in_maps


def _build(variant="full", num_devices=N_CORES):
    import concourse.bacc as bacc
    import concourse.mybir as mybir
    from concourse import tile

    dt = mybir.dt
    alu = mybir.AluOpType

    nc = bacc.Bacc("TRN2", target_bir_lowering=False, debug=False,
                   num_devices=num_devices, num_swdge_queues=4)

    ids_s_d = nc.dram_tensor("ids_s", [128, NBLK], dt.bfloat16,
                             kind="ExternalInput")
    ids_d_d = nc.dram_tensor("ids_d", [128, NBLK], dt.bfloat16,
                             kind="ExternalInput")
    xshard_d = nc.dram_tensor("xshard", [NT * 128, F], dt.bfloat16,
                              kind="ExternalInput")
    ndst_d = nc.dram_tensor("ndst", [128, NT], dt.float32, kind="ExternalInput")
    nso_d = nc.dram_tensor("nso", [128, NT], dt.float32, kind="ExternalInput")
    w1_d = nc.dram_tensor("w1", [128, 2 * H], dt.bfloat16, kind="ExternalInput")
    w2_d = nc.dram_tensor("w2", [128, 2 * C], dt.bfloat16, kind="ExternalInput")
    b1_d = nc.dram_tensor("b1", [1, H], dt.float32, kind="ExternalInput")
    b2_d = nc.dram_tensor("b2", [1, C], dt.float32, kind="ExternalInput")
    out_d = nc.dram_tensor("out", [NT * 128, C], dt.float32,
                           kind="ExternalOutput")

    with tile.TileContext(nc) as tc:
        with (
            tc.tile_pool(name="const", bufs=1) as const,
            tc.tile_pool(name="dram", bufs=1, space="DRAM") as dram,
            tc.tile_pool(name="oh", bufs=2) as ohpool,
            tc.tile_pool(name="asb", bufs=3) as asb,
            tc.tile_pool(name="work", bufs=3) as work,
            tc.tile_pool(name="ps_agg", bufs=2, space="PSUM") as ps_agg,
            tc.tile_pool(name="ps_ba", bufs=2, space="PSUM") as ps_ba,
            tc.tile_pool(name="ps_tr", bufs=2, space="PSUM") as ps_tr,
            tc.tile_pool(name="ps_h", bufs=1, space="PSUM") as ps_h,
        ):
            # ---- constants ----
            ids_s = const.tile([128, NBLK], dt.bfloat16)
            nc.sync.dma_start(ids_s[:], ids_s_d.ap())
            ids_d = const.tile([128, NBLK], dt.bfloat16)
            nc.sync.dma_start(ids_d[:], ids_d_d.ap())
            ndst_sb = const.tile([128, NT], dt.float32)
            nc.sync.dma_start(ndst_sb[:], ndst_d.ap())
            nso_sb = const.tile([128, NT], dt.float32)
            nc.sync.dma_start(nso_sb[:], nso_d.ap())
            w1_sb = const.tile([128, 2 * H], dt.bfloat16)
            nc.sync.dma_start(w1_sb[:], w1_d.ap())
            w2_sb = const.tile([128, 2 * C], dt.bfloat16)
            nc.sync.dma_start(w2_sb[:], w2_d.ap())
            b1_sb = const.tile([1, H], dt.float32)
            nc.sync.dma_start(b1_sb[:], b1_d.ap())
            b2_sb = const.tile([1, C], dt.float32)
            nc.sync.dma_start(b2_sb[:], b2_d.ap())

            # iota3[p, s, j] = j  (for one-hot builds over a whole window)
            iota3 = const.tile([128, KS, 128], dt.bfloat16)
            nc.gpsimd.iota(iota3[:], pattern=[[0, KS], [1, 128]], base=0,
                           channel_multiplier=0,
                           allow_small_or_imprecise_dtypes=True)
            iota_col = const.tile([128, 1], dt.float32)
            nc.gpsimd.iota(iota_col[:], pattern=[[0, 1]], base=0,
                           channel_multiplier=1,
                           allow_small_or_imprecise_dtypes=True)
            ident_bf = const.tile([128, 128], dt.bfloat16)
            nc.vector.tensor_scalar(ident_bf[:], iota3[:, 0, :], iota_col[:],
                                    None, alu.is_equal)

            ones_sb = const.tile([1, 128], dt.float32)
            nc.vector.memset(ones_sb[:], 1.0)
            b1_ps = ps_h.tile([128, H], dt.float32, tag="h")
            nc.tensor.matmul(b1_ps[:], lhsT=ones_sb[:], rhs=b1_sb[:],
                             start=True, stop=True)
            b1_bc = const.tile([128, H], dt.float32)
            nc.vector.tensor_copy(b1_bc[:], b1_ps[:])
            b2_ps = ps_h.tile([128, C], dt.float32, tag="zn")
            nc.tensor.matmul(b2_ps[:], lhsT=ones_sb[:], rhs=b2_sb[:],
                             start=True, stop=True)
            b2_bc = const.tile([128, C], dt.float32)
            nc.vector.tensor_copy(b2_bc[:], b2_ps[:])

            xsb = const.tile([128, KS, F], dt.bfloat16)
            zsb = const.tile([128, KS, C], dt.bfloat16)
            zloc = const.tile([128, NT, C], dt.bfloat16)

            cc_xin = dram.tile([NT * 128, F], dt.bfloat16)
            x_full = dram.tile([N_CORES * NT * 128, F], dt.bfloat16,
                               addr_space="Shared")
            cc_zin = dram.tile([NT * 128, C], dt.bfloat16)
            z_full = dram.tile([N_CORES * NT * 128, C], dt.bfloat16,
                               addr_space="Shared")

            # ---- gather the feature table ----
            xtmp = const.tile([128, NT, F], dt.bfloat16)
            nc.sync.dma_start(
                xtmp[:], xshard_d.ap().rearrange("(p k) f -> p k f", k=NT))
            nc.sync.dma_start(
                cc_xin[:, :].rearrange("(p k) f -> p k f", k=NT), xtmp[:])
            if variant == "no_cc" or num_devices == 1:
                nc.sync.dma_start(x_full[0:NT * 128, :], cc_xin[:, :])
            else:
                nc.gpsimd.collective_compute(
                    "AllGather", alu.bypass,
                    replica_groups=[list(range(N_CORES))],
                    ins=[cc_xin.opt()], outs=[x_full.opt()])
            for c in range(N_CORES):
                nc.sync.dma_start(
                    xsb[:, c * NT:(c + 1) * NT, :],
                    x_full[c * NT * 128:(c + 1) * NT * 128, :]
                    .rearrange("(p k) f -> p k f", k=NT))

            def agg_layer(w, rhs_sb, width, ps_out, tag):
                """One dst window's aggregation: returns PSUM [128, width]."""
                ohS = ohpool.tile([128, KS, 128], dt.bfloat16, tag="ohS")
                nc.vector.tensor_tensor(
                    ohS[:], iota3[:],
                    ids_s[:, w * KS:(w + 1) * KS].broadcast_to((128, KS, 128)),
                    alu.is_equal)
                ohD = ohpool.tile([128, KS, 128], dt.bfloat16, tag="ohD")
                nc.vector.tensor_tensor(
                    ohD[:], iota3[:],
                    ids_d[:, w * KS:(w + 1) * KS].broadcast_to((128, KS, 128)),
                    alu.is_equal)
                agg = ps_out.tile([128, width], dt.float32, tag=tag)
                for q in range(KS // 4):
                    ba = ps_ba.tile([128, 4, 128], dt.float32, tag="ba")
                    for j in range(4):
                        s = q * 4 + j
                        nc.tensor.matmul(ba[:, j, :], lhsT=ohS[:, s, :],
                                         rhs=ohD[:, s, :],
                                         start=True, stop=True)
                    aT = asb.tile([128, 4, 128], dt.bfloat16, tag="aT")
                    if q % 2 == 0:
                        nc.vector.tensor_copy(aT[:], ba[:])
                    else:
                        nc.scalar.activation(
                            aT[:], ba[:], mybir.ActivationFunctionType.Copy)
                    for j in range(4):
                        s = q * 4 + j
                        nc.tensor.matmul(agg[:], lhsT=aT[:, j, :],
                                         rhs=rhs_sb[:, s, 0:width],
                                         start=(s == 0), stop=(s == KS - 1))
                return agg

            # ---- layer 1 ----
            for w in range(NT if variant != "nothing" else 0):
                agg = agg_layer(w, xsb, F, ps_agg, "agg")
                aggc = work.tile([128, F], dt.bfloat16, tag="aggc")
                nc.vector.tensor_copy(aggc[:], agg[:])
                aggT = work.tile([128, 2, 128], dt.bfloat16, tag="aggT")
                for k in range(2):
                    trp = ps_tr.tile([128, 128], dt.bfloat16, tag="tr")
                    nc.tensor.transpose(trp[:], aggc[:, k * 128:(k + 1) * 128],
                                        ident_bf[:])
                    nc.vector.tensor_copy(aggT[:, k, :], trp[:])
                h1 = ps_h.tile([128, H], dt.float32, tag="h")
                for k in range(2):
                    nc.tensor.matmul(h1[:], lhsT=aggT[:, k, :],
                                     rhs=w1_sb[:, k * H:(k + 1) * H],
                                     start=(k == 0), stop=(k == 1))
                t1 = work.tile([128, H], dt.float32, tag="t1")
                nc.vector.scalar_tensor_tensor(t1[:], h1[:], ndst_sb[:, w:w + 1],
                                               b1_bc[:], alu.mult, alu.add)
                yz = work.tile([128, H], dt.bfloat16, tag="yz")
                nc.scalar.activation(yz[:], t1[:],
                                     mybir.ActivationFunctionType.Relu,
                                     scale=nso_sb[:, w:w + 1])
                yzT = work.tile([128, 2, 128], dt.bfloat16, tag="yzT")
                for k in range(2):
                    trp2 = ps_tr.tile([128, 128], dt.bfloat16, tag="tr")
                    nc.tensor.transpose(trp2[:], yz[:, k * 128:(k + 1) * 128],
                                        ident_bf[:])
                    nc.vector.tensor_copy(yzT[:, k, :], trp2[:])
                zn = ps_h.tile([128, C], dt.float32, tag="zn")
                for k in range(2):
                    nc.tensor.matmul(zn[:], lhsT=yzT[:, k, :],
                                     rhs=w2_sb[:, k * C:(k + 1) * C],
                                     start=(k == 0), stop=(k == 1))
                nc.vector.tensor_copy(zloc[:, w, :], zn[:])

            # ---- halo exchange (z) ----
            if variant == "nothing":
                nc.vector.memset(zsb[:], 0.0)
            else:
                nc.sync.dma_start(
                    cc_zin[:, :].rearrange("(p k) n -> p k n", k=NT), zloc[:])
                if variant == "no_cc" or num_devices == 1:
                    nc.sync.dma_start(z_full[0:NT * 128, :], cc_zin[:, :])
                else:
                    nc.gpsimd.collective_compute(
                        "AllGather", alu.bypass,
                        replica_groups=[list(range(N_CORES))],
                        ins=[cc_zin.opt()], outs=[z_full.opt()])
                for c in range(N_CORES):
                    nc.sync.dma_start(
                        zsb[:, c * NT:(c + 1) * NT, :],
                        z_full[c * NT * 128:(c + 1) * NT * 128, :]
                        .rearrange("(p k) n -> p k n", k=NT))

            # ---- layer 2 ----
            for w in range(NT if variant != "nothing" else 1):
                agg2 = agg_layer(w, zsb, C, ps_agg, "agg")
                ot = work.tile([128, C], dt.float32, tag="ot")
                nc.vector.scalar_tensor_tensor(ot[:], agg2[:],
                                               ndst_sb[:, w:w + 1], b2_bc[:],
                                               alu.mult, alu.add)
                nc.sync.dma_start(out_d.ap()[w * 128:(w + 1) * 128, :], ot[:])

    nc.compile()
    return nc


def kernel(features, W1, b1, W2, b2, src, dst, **_):
    import time
    from concourse.bass_utils import run_bass_kernel_spmd

    t0 = time.time()
    in_maps = _prep(features, W1, b1, W2, b2, src, dst)
    t1 = time.time()
    nc = _build()
    t2 = time.time()
    res = run_bass_kernel_spmd(nc, in_maps, core_ids=list(range(N_CORES)))
    t3 = time.time()
    out = np.concatenate([res.results[c]["out"][:NPC] for c in range(N_CORES)], 0)
    LAST_STATS.update(prep_s=t1 - t0, build_s=t2 - t1, run_s=t3 - t2)
    return np.ascontiguousarray(out.astype(np.float32))
